# revision 5
# baseline (speedup 1.0000x reference)
"""ArcDecoder distributed Bass kernel for 8 TRN2 NeuronCores.

Problem: for each arc e with endpoints (s, d):
    h   = concat(z[s], z[d])                # [256]
    h1  = relu(W1 @ h + b1)                 # [128]
    out = W2 @ h1 + b2                      # scalar

Strategy (dense, host-expanded, fp8 DoubleRowSwInterleave): the host
pre-gathers the endpoint embeddings into one interleaved stream per core
in fp8-e4m3, laid out for MatmulPerfMode.DoubleRowSwInterleave (pairs
(zs, zd) interleaved per arc, arcs reversed within each 128-slot tile).
W1 is folded with |W2| and scaled by a power-of-2 S so fp8 quantization
avoids subnormal underflow; j-columns are ordered
[P-zone asc|w2| | N-zone desc|w2|] (k = #positives) so the sign zones
are contiguous and the top-m columns by |w2| form one contiguous
mid-block [k-mP, k+mN) which gets an fp8 residual correction
r8 = fp8(S*w - w8) streamed as a SECOND small matmul.

Per tile, ONE LDWEIGHTS + TWO matmuls (residual first):
    MM1: psum[arc, c0:c0+m] = z_cat.T @ r8   (start=True, N=m small)
    MM2: psum[arc, 0:128]  += z_cat.T @ w8   (stop=True,  N=128)
Both share the same stationary z_cat tile; a post-legalization pass
removes the duplicate LDWEIGHTS (bass emits one per matmul), which
restores the 1-LDW pipeline (~84ns/tile vs 213 with 2 LDWs).

Post-matmul, split across engines (psum chunk = 16 tiles, rs
super-tile = 32 tiles so GPSIMD/DVE instructions amortize overheads):
    relu on ACT (j in [0:X_ACT)) and DVE (rest) -> rs bf16,
    GPSIMD folds each zone's upper half onto its lower half,
    DVE segmented add-reduce of folded halves -> gP, gN (bf16),
    host: out = (gP - gN)/S + b2.
Fully dense streaming: 1MB DMAs, no gathers, no collectives.
"""

import math

import numpy as np

# ---------------- problem constants (hardcoded, per the task spec) ----------
N_NODES = 100000
HIDDEN = 128
N_ARCS = 1000000
N_CORES = 8

P = 128  # SBUF partitions

E_PER_CORE = N_ARCS // N_CORES  # 125000
NT = 992                        # slot tiles per core (992*128 = 126976 slots)
E_PAD = NT * P

PCHUNK = 8    # tiles per psum chunk (8*128 f32 = 4KB/partition = 2 banks)
SUPER = 4     # psum chunks per rs super-tile (32 tiles)
DCHUNK = 32   # tiles per input-DMA chunk (32*256*128 = 1MB fp8)
N_DC = NT // DCHUNK   # 31
N_CH = NT // PCHUNK   # 62

M_RES = 64    # residual-corrected columns (top |w2|), contiguous mid-block
X_ACT = 114   # relu j-split: ACT does [0:x), DVE does [x:128)
FOLD_GPS = 22 # of the 32 super tiles, GPSIMD folds [0:22), DVE folds [22:32)


def _strip_duplicate_ldweights(nc):
    """Remove InstLdweights that reload the exact weights AP the PE already
    holds (consecutive matmuls sharing one stationary tile). bass emits one
    LDWEIGHTS per InstMatmult at legalization; the matmuls themselves are
    non-self-loading, so the duplicate load is pure overhead (~130ns/tile).
    """
    removed = 0
    for blk in nc.main_func.blocks:
        insts = list(blk.instructions)
        keep, last_sig = [], None
        for i in insts:
            tn = type(i).__name__
            if tn == "InstLdweights":
                w = i.ins[0]
                sig = (str(w), getattr(w, "offset", None))
                if sig == last_sig:
                    removed += 1
                    continue
                last_sig = sig
            elif tn != "InstMatmult":
                last_sig = None
            keep.append(i)
        if len(keep) != len(insts):
            blk.instructions = keep
    return removed


def _build_graph(k_pos, m_p, m_n):
    """Build the SPMD single-core graph (all 8 cores run this same graph).

    k_pos: number of leading j-columns with sgn(W2) = +1 (rest negative).
    m_p, m_n: residual block spans psum columns [k_pos-m_p, k_pos+m_n).
    """
    import concourse.bass as bass
    from concourse import bacc, mybir, tile

    BF16 = mybir.dt.bfloat16
    F32 = mybir.dt.float32
    E4 = mybir.dt.float8e4
    DRS = mybir.MatmulPerfMode.DoubleRowSwInterleave

    k = k_pos
    m = m_p + m_n
    c0 = k - m_p  # residual block start column
    hp = (k + 1) // 2
    hn = (P - k + 1) // 2
    x = X_ACT
    SPC = SUPER * PCHUNK  # tiles per rs super-tile

    nc = bacc.Bacc(None, target_bir_lowering=False)
    with tile.TileContext(nc) as tc:
        with tc.tile_pool(name="dram", bufs=1, space="DRAM") as dram:
            zc_d = dram.tile([P, NT * 2 * P], E4, kind="ExternalInput",
                             name="zc", uniquify=False)
            w8_d = dram.tile([P, 2 * P], E4, kind="ExternalInput",
                             name="w8", uniquify=False)
            r8_d = dram.tile([P, 2 * max(m, 1)], E4, kind="ExternalInput",
                             name="r8", uniquify=False)
            outm = dram.tile([P, 2 * NT], BF16, kind="ExternalOutput",
                             name="outm", uniquify=False)

            with tc.tile_pool(name="consts", bufs=1) as cpool:
                w8_s = cpool.tile([P, 2, P], E4, name="w8_s")
                nc.sync.dma_start(
                    out=w8_s[:].rearrange("p a b -> p (a b)"), in_=w8_d[:])
                r8_s = cpool.tile([P, 2, max(m, 1)], E4, name="r8_s")
                nc.sync.dma_start(
                    out=r8_s[:].rearrange("p a b -> p (a b)"), in_=r8_d[:])
                GCH = 4  # supers per result-flush group (4*32=128 cols)

                with tc.tile_pool(name="zin", bufs=3) as zpool, \
                     tc.tile_pool(name="ps", bufs=4, space="PSUM") as pspool, \
                     tc.tile_pool(name="res", bufs=2) as respool, \
                     tc.tile_pool(name="rs", bufs=4) as rspool:
                    gP = gN = rs = None
                    n_sup = N_CH // SUPER
                    for c2 in range(N_DC):
                        zc_t = zpool.tile([P, DCHUNK, P, 2], E4, tag="zc")
                        nc.sync.dma_start(
                            out=zc_t[:].rearrange("p t a b -> p (t a b)"),
                            in_=zc_d[:, c2 * DCHUNK * 2 * P:
                                     (c2 + 1) * DCHUNK * 2 * P])
                        for h in range(DCHUNK // PCHUNK):
                            c = c2 * (DCHUNK // PCHUNK) + h
                            sup, ci = divmod(c, SUPER)
                            ps = pspool.tile([P, PCHUNK, P], F32, tag="ps")
                            for t in range(PCHUNK):
                                tt = h * PCHUNK + t
                                zt = zc_t[:, tt, :, :].rearrange(
                                    "p a b -> p (a b)")
                                if m > 0:
                                    nc.tensor.matmul(ps[:, t, c0:c0 + m],
                                                     lhsT=zt, rhs=r8_s[:],
                                                     start=True, stop=False,
                                                     perf_mode=DRS,
                                                     skip_group_check=True)
                                nc.tensor.matmul(ps[:, t, :],
                                                 lhsT=zt, rhs=w8_s[:],
                                                 start=(m == 0), stop=True,
                                                 perf_mode=DRS,
                                                 skip_group_check=True)
                            # relu: ACT on [0:x), DVE on [x:128)
                            if ci == 0:
                                rs = rspool.tile([P, SPC, P], BF16, tag="rs")
                            rsv = rs[:, ci * PCHUNK:(ci + 1) * PCHUNK, :]
                            nc.scalar.activation(
                                out=rsv[:, :, 0:x],
                                in_=ps[:, :, 0:x],
                                func=mybir.ActivationFunctionType.Relu)
                            if x < P:
                                nc.vector.tensor_scalar_max(
                                    rsv[:, :, x:P], ps[:, :, x:P], 0.0)
                            if ci != SUPER - 1:
                                continue
                            # per super-tile: folds (GPSIMD) + reduces (DVE)
                            fg = FOLD_GPS
                            for eng, t0, t1 in ((nc.gpsimd, 0, fg),
                                                (nc.vector, fg, SPC)):
                                if k > 1:
                                    eng.tensor_tensor(
                                        out=rs[:, t0:t1, 0:k - hp],
                                        in0=rs[:, t0:t1, 0:k - hp],
                                        in1=rs[:, t0:t1, hp:k],
                                        op=mybir.AluOpType.add)
                                if P - k > 1:
                                    eng.tensor_tensor(
                                        out=rs[:, t0:t1, k:P - hn],
                                        in0=rs[:, t0:t1, k:P - hn],
                                        in1=rs[:, t0:t1, k + hn:P],
                                        op=mybir.AluOpType.add)
                            if sup % GCH == 0:
                                gW = min(GCH, n_sup - sup) * SPC
                                gP = respool.tile([P, GCH * SPC], BF16,
                                                  tag="gP")
                                gN = respool.tile([P, GCH * SPC], BF16,
                                                  tag="gN")
                                if k == 0:
                                    nc.vector.memset(gP[:], 0.0)
                                if k == P:
                                    nc.vector.memset(gN[:], 0.0)
                            o0 = (sup % GCH) * SPC
                            with nc.allow_low_precision("bf16 partials"):
                                if k > 0:
                                    nc.vector.tensor_reduce(
                                        out=gP[:, o0:o0 + SPC],
                                        in_=rs[:, :, 0:hp],
                                        axis=mybir.AxisListType.X,
                                        op=mybir.AluOpType.add)
                                if k < P:
                                    nc.vector.tensor_reduce(
                                        out=gN[:, o0:o0 + SPC],
                                        in_=rs[:, :, k:k + hn],
                                        axis=mybir.AxisListType.X,
                                        op=mybir.AluOpType.add)
                            if sup % GCH == GCH - 1 or sup == n_sup - 1:
                                gb = (sup // GCH) * GCH * SPC
                                nc.sync.dma_start(
                                    out=outm[:, gb:gb + gW],
                                    in_=gP[:, 0:gW])
                                nc.sync.dma_start(
                                    out=outm[:, NT + gb:NT + gb + gW],
                                    in_=gN[:, 0:gW])
    _strip_duplicate_ldweights(nc)
    nc.compile()
    return nc


def _prep_weights(W1, b1, W2, m_res=M_RES):
    """Fold |W2| into W1, order j-columns, scale, quantize to fp8 + residual.

    Returns (w8 [128,2,128], r8 [128,2,m], k_pos, m_p, m_n, S, order).
    j order: [P-zone asc |w2| | N-zone desc |w2|] -> top-m block is the
    contiguous mid-range [k-mP, k+mN).
    """
    import ml_dtypes

    e4 = ml_dtypes.float8_e4m3
    H = HIDDEN
    W1 = np.asarray(W1, np.float32)
    W2 = np.asarray(W2, np.float32).reshape(-1)
    absw2 = np.abs(W2)
    sgn = np.sign(W2)

    pos = np.nonzero(sgn >= 0)[0]
    neg = np.nonzero(sgn < 0)[0]
    pos = pos[np.argsort(absw2[pos], kind="stable")]          # asc
    neg = neg[np.argsort(-absw2[neg], kind="stable")]         # desc
    order = np.concatenate([pos, neg])
    k_pos = len(pos)

    m = min(m_res, HIDDEN)
    if m > 0:
        thresh_idx = np.argsort(-absw2)[:m]
        in_top = np.zeros(HIDDEN, bool)
        in_top[thresh_idx] = True
        m_p = int(in_top[pos].sum())
        m_n = m - m_p
        # keep the block inside [0, 128)
        if k_pos - m_p < 0:
            m_p = k_pos
            m_n = m - m_p
        if k_pos + m_n > HIDDEN:
            m_n = HIDDEN - k_pos
            m_p = m - m_n
    else:
        m_p = m_n = 0

    wa = (W1[:, :H] * absw2[:, None]).T[:, order]   # [128 in, 128 j]
    wb = (W1[:, H:] * absw2[:, None]).T[:, order]
    wcat = np.stack([wa, wb], 0)                    # [2, 128 in, 128 j]
    wmax = float(np.abs(wcat).max())
    S = 2.0 ** math.floor(math.log2(224.0 / max(wmax, 1e-30)))
    w_s = (wcat * S).astype(np.float32)
    w8 = w_s.astype(e4)
    r_f = w_s - w8.astype(np.float32)
    c0 = k_pos - m_p
    r8 = r_f[:, :, c0:c0 + max(m, 1)].astype(e4)
    # engine layout: [128 in-part, 2 slab, 128 j]
    w8 = np.ascontiguousarray(w8.transpose(1, 0, 2))
    r8 = np.ascontiguousarray(r8.transpose(1, 0, 2))
    return w8, r8, k_pos, m_p, m_n, S, order


def _host_prep(z, pot_arcs, W1, b1, W2, b2, m_res=M_RES):
    """Stage inputs: fold/scale/quantize weights, expand+interleave z."""
    import ml_dtypes

    e4 = ml_dtypes.float8_e4m3
    z = np.asarray(z, np.float32)
    b1 = np.asarray(b1, np.float32).reshape(-1)
    b2 = np.asarray(b2, np.float32).reshape(-1)
    arcs = np.asarray(pot_arcs)
    assert not np.any(b1), "b1 folding not implemented (b1 == 0 in this task)"

    w8, r8, k_pos, m_p, m_n, S, order = _prep_weights(W1, b1, W2, m_res)
    w8_flat = w8.reshape(P, -1)
    r8_flat = r8.reshape(P, -1)

    zT = np.ascontiguousarray(z.T)  # [128, N] f32
    src = np.asarray(arcs[:, 0], np.int64)
    dst = np.asarray(arcs[:, 1], np.int64)
    in_maps = []
    for ci in range(N_CORES):
        lo, hi = ci * E_PER_CORE, (ci + 1) * E_PER_CORE
        s_idx = np.zeros(E_PAD, np.int64)
        d_idx = np.zeros(E_PAD, np.int64)
        s_idx[:E_PER_CORE] = src[lo:hi]
        d_idx[:E_PER_CORE] = dst[lo:hi]
        zs = zT[:, s_idx].astype(e4).reshape(P, NT, P)
        zd = zT[:, d_idx].astype(e4).reshape(P, NT, P)
        # SwInterleave stationary layout: [p, t, arc_rev, (zs, zd)]
        zc = np.stack([zs[:, :, ::-1], zd[:, :, ::-1]], axis=3)
        in_maps.append(dict(
            zc=np.ascontiguousarray(zc).reshape(P, NT * 2 * P),
            w8=w8_flat, r8=r8_flat))
    return in_maps, float(b2[0]), k_pos, m_p, m_n, S


def _assemble(results, b2_val, S):
    """results[c]["outm"] is [128, 2*NT] bf16: [gP | gN] columns."""
    out = np.empty(N_ARCS, np.float32)
    for c in range(N_CORES):
        dev = np.asarray(results[c]["outm"], np.float32)
        val = dev[:, :NT] - dev[:, NT:]
        out[c * E_PER_CORE:(c + 1) * E_PER_CORE] = \
            val.T.reshape(-1)[:E_PER_CORE]
    return out / S + b2_val


_GRAPH_CACHE = {}


def _get_graph(k_pos, m_p, m_n):
    key = (k_pos, m_p, m_n)
    if key not in _GRAPH_CACHE:
        _GRAPH_CACHE[key] = _build_graph(k_pos, m_p, m_n)
    return _GRAPH_CACHE[key]


def _pick_m_res(z, pot_arcs, W1, b1, W2, b2, sample=100000):
    """Host-side emulation of device numerics on an arc sample. Returns the
    smallest residual width m_res clearing the 2e-2 gate with margin.
    """
    import ml_dtypes

    e4 = ml_dtypes.float8_e4m3
    z32 = np.asarray(z, np.float32)
    W1f = np.asarray(W1, np.float32)
    W2f = np.asarray(W2, np.float32).reshape(-1)
    arcs = np.asarray(pot_arcs)[:sample]
    s, d = arcs[:, 0].astype(np.int64), arcs[:, 1].astype(np.int64)
    zs8 = z32[s].astype(e4).astype(np.float32)
    zd8 = z32[d].astype(e4).astype(np.float32)
    h = np.concatenate([z32[s], z32[d]], -1)
    exact = (np.maximum(h @ W1f.T, 0) @ W2f).reshape(-1)
    nex = max(np.linalg.norm(exact), 1e-30)
    sgn = np.sign(W2f)

    for m in (M_RES, 128):
        w8, r8, k_pos, m_p, m_n, S, order = _prep_weights(W1, b1, W2, m)
        wtot = w8.astype(np.float32)            # [128, 2, 128]
        c0 = k_pos - m_p
        if m_p + m_n > 0:
            wtot[:, :, c0:c0 + m_p + m_n] += r8.astype(np.float32)
        q = zs8 @ wtot[:, 0, :] + zd8 @ wtot[:, 1, :]
        rs = np.maximum(q, 0).astype(ml_dtypes.bfloat16).astype(np.float32)
        sg = np.where(sgn[order] >= 0, 1.0, -1.0).astype(np.float32)
        out = (rs * sg).sum(1) / S
        rel = np.linalg.norm(out - exact) / nex
        if rel < 1.9e-2:
            return m
    return 128


def kernel(z, pot_arcs, W1, b1, W2, b2):
    from concourse.bass_utils import run_bass_kernel_spmd

    m_res = _pick_m_res(z, pot_arcs, W1, b1, W2, b2)
    in_maps, b2_val, k_pos, m_p, m_n, S = _host_prep(
        z, pot_arcs, W1, b1, W2, b2, m_res=m_res)
    nc = _get_graph(k_pos, m_p, m_n)
    res = run_bass_kernel_spmd(nc, in_maps, core_ids=list(range(N_CORES)))
    return _assemble(res.results, b2_val, S)


# revision 7
# speedup vs baseline: 1.0605x; 1.0605x over previous
"""ArcDecoder distributed Bass kernel for 8 TRN2 NeuronCores.

Problem: for each arc e with endpoints (s, d):
    h   = concat(z[s], z[d])                # [256]
    h1  = relu(W1 @ h + b1)                 # [128]
    out = W2 @ h1 + b2                      # scalar

Strategy (dense, host-expanded, fp8 DoubleRowSwInterleave): the host
pre-gathers the endpoint embeddings into one interleaved stream per core
in fp8-e4m3, laid out for MatmulPerfMode.DoubleRowSwInterleave (pairs
(zs, zd) interleaved per arc, arcs reversed within each 128-slot tile).
W1 is folded with |W2| and scaled by a power-of-2 S so fp8 quantization
avoids subnormal underflow; j-columns are ordered
[P-zone asc|w2| | N-zone desc|w2|] (k = #positives) so the sign zones
are contiguous and the top-m columns by |w2| form one contiguous
mid-block [k-mP, k+mN) which gets an fp8 residual correction
r8 = fp8(S*w - w8) streamed as a SECOND small matmul.

Per tile, ONE LDWEIGHTS + TWO matmuls (residual first):
    MM1: psum[arc, c0:c0+m] = z_cat.T @ r8   (start=True, N=m small)
    MM2: psum[arc, 0:128]  += z_cat.T @ w8   (stop=True,  N=128)
Both share the same stationary z_cat tile; a post-legalization pass
removes the duplicate LDWEIGHTS (bass emits one per matmul), which
restores the 1-LDW pipeline (~84ns/tile vs 213 with 2 LDWs).

Post-matmul, split across engines (psum chunk = 16 tiles, rs
super-tile = 32 tiles so GPSIMD/DVE instructions amortize overheads):
    relu on ACT (j in [0:X_ACT)) and DVE (rest) -> rs bf16,
    GPSIMD folds each zone's upper half onto its lower half,
    DVE segmented add-reduce of folded halves -> gP, gN (bf16),
    host: out = (gP - gN)/S + b2.
Fully dense streaming: 1MB DMAs, no gathers, no collectives.
"""

import math

import numpy as np

# ---------------- problem constants (hardcoded, per the task spec) ----------
N_NODES = 100000
HIDDEN = 128
N_ARCS = 1000000
N_CORES = 8

P = 128  # SBUF partitions

E_PER_CORE = N_ARCS // N_CORES  # 125000
NT = 992                        # slot tiles per core (992*128 = 126976 slots)
E_PAD = NT * P

PCHUNK = 16   # tiles per psum chunk (16*128 f32 = 8KB/partition = 4 banks)
SUPER = 2     # psum chunks per rs super-tile (32 tiles)
DCHUNK = 32   # tiles per input-DMA chunk (32*256*128 = 1MB fp8)
N_DC = NT // DCHUNK   # 31
N_CH = NT // PCHUNK   # 62

M_RES = 64    # residual-corrected columns (top |w2|), contiguous mid-block
X_ACT = 114   # relu j-split: ACT does [0:x), DVE does [x:128)
FOLD_GPS = 21 # of the 32 super tiles, GPSIMD folds [0:21), DVE folds [21:32)


def _strip_duplicate_ldweights(nc):
    """Remove InstLdweights that reload the exact weights AP the PE already
    holds (consecutive matmuls sharing one stationary tile). bass emits one
    LDWEIGHTS per InstMatmult at legalization; the matmuls themselves are
    non-self-loading, so the duplicate load is pure overhead (~130ns/tile).
    """
    removed = 0
    for blk in nc.main_func.blocks:
        insts = list(blk.instructions)
        keep, last_sig = [], None
        for i in insts:
            tn = type(i).__name__
            if tn == "InstLdweights":
                w = i.ins[0]
                sig = (str(w), getattr(w, "offset", None))
                if sig == last_sig:
                    removed += 1
                    continue
                last_sig = sig
            elif tn != "InstMatmult":
                last_sig = None
            keep.append(i)
        if len(keep) != len(insts):
            blk.instructions = keep
    return removed


def _build_graph(k_pos, m_p, m_n):
    """Build the SPMD single-core graph (all 8 cores run this same graph).

    k_pos: number of leading j-columns with sgn(W2) = +1 (rest negative).
    m_p, m_n: residual block spans psum columns [k_pos-m_p, k_pos+m_n).
    """
    import concourse.bass as bass
    from concourse import bacc, mybir, tile

    BF16 = mybir.dt.bfloat16
    F32 = mybir.dt.float32
    E4 = mybir.dt.float8e4
    DRS = mybir.MatmulPerfMode.DoubleRowSwInterleave

    k = k_pos
    m = m_p + m_n
    c0 = k - m_p  # residual block start column
    hp = (k + 1) // 2
    hn = (P - k + 1) // 2
    x = X_ACT
    SPC = SUPER * PCHUNK  # tiles per rs super-tile

    nc = bacc.Bacc(None, target_bir_lowering=False)
    with tile.TileContext(nc) as tc:
        with tc.tile_pool(name="dram", bufs=1, space="DRAM") as dram:
            zc_d = dram.tile([P, NT * 2 * P], E4, kind="ExternalInput",
                             name="zc", uniquify=False)
            w8_d = dram.tile([P, 2 * P], E4, kind="ExternalInput",
                             name="w8", uniquify=False)
            r8_d = dram.tile([P, 2 * max(m, 1)], E4, kind="ExternalInput",
                             name="r8", uniquify=False)
            outm = dram.tile([P, NT], BF16, kind="ExternalOutput",
                             name="outm", uniquify=False)

            with tc.tile_pool(name="consts", bufs=1) as cpool:
                w8_s = cpool.tile([P, 2, P], E4, name="w8_s")
                nc.sync.dma_start(
                    out=w8_s[:].rearrange("p a b -> p (a b)"), in_=w8_d[:])
                r8_s = cpool.tile([P, 2, max(m, 1)], E4, name="r8_s")
                nc.sync.dma_start(
                    out=r8_s[:].rearrange("p a b -> p (a b)"), in_=r8_d[:])
                GCH = 4  # supers per result-flush group (4*32=128 cols)

                with tc.tile_pool(name="zin", bufs=3) as zpool, \
                     tc.tile_pool(name="ps", bufs=2, space="PSUM") as pspool, \
                     tc.tile_pool(name="res", bufs=2) as respool, \
                     tc.tile_pool(name="rs", bufs=4) as rspool:
                    gP = gN = rs = None
                    n_sup = N_CH // SUPER
                    for c2 in range(N_DC):
                        zc_t = zpool.tile([P, DCHUNK, P, 2], E4, tag="zc")
                        nc.sync.dma_start(
                            out=zc_t[:].rearrange("p t a b -> p (t a b)"),
                            in_=zc_d[:, c2 * DCHUNK * 2 * P:
                                     (c2 + 1) * DCHUNK * 2 * P])
                        for h in range(DCHUNK // PCHUNK):
                            c = c2 * (DCHUNK // PCHUNK) + h
                            sup, ci = divmod(c, SUPER)
                            ps = pspool.tile([P, PCHUNK, P], F32, tag="ps")
                            for t in range(PCHUNK):
                                tt = h * PCHUNK + t
                                zt = zc_t[:, tt, :, :].rearrange(
                                    "p a b -> p (a b)")
                                if m > 0:
                                    nc.tensor.matmul(ps[:, t, c0:c0 + m],
                                                     lhsT=zt, rhs=r8_s[:],
                                                     start=True, stop=False,
                                                     perf_mode=DRS,
                                                     skip_group_check=True)
                                nc.tensor.matmul(ps[:, t, :],
                                                 lhsT=zt, rhs=w8_s[:],
                                                 start=(m == 0), stop=True,
                                                 perf_mode=DRS,
                                                 skip_group_check=True)
                            # relu: ACT on [0:x), DVE on [x:128)
                            if ci == 0:
                                rs = rspool.tile([P, SPC, P], BF16, tag="rs")
                            rsv = rs[:, ci * PCHUNK:(ci + 1) * PCHUNK, :]
                            nc.scalar.activation(
                                out=rsv[:],
                                in_=ps[:],
                                func=mybir.ActivationFunctionType.Relu)
                            if ci != SUPER - 1:
                                continue
                            # per super-tile: folds (GPSIMD) + reduces (DVE)
                            u = P - k - hn  # N-zone upper-half width
                            hp2 = (hp + 1) // 2
                            # GPSIMD: P-zone folds, two levels (plain adds)
                            if k > 1:
                                nc.gpsimd.tensor_tensor(
                                    out=rs[:, :, 0:k - hp],
                                    in0=rs[:, :, 0:k - hp],
                                    in1=rs[:, :, hp:k],
                                    op=mybir.AluOpType.add)
                            if hp > 1:
                                nc.gpsimd.tensor_tensor(
                                    out=rs[:, :, 0:hp - hp2],
                                    in0=rs[:, :, 0:hp - hp2],
                                    in1=rs[:, :, hp2:hp],
                                    op=mybir.AluOpType.add)
                            # DVE: N-zone negating fold into [hp2, hp2+hn)
                            if u > 0:
                                nc.vector.scalar_tensor_tensor(
                                    out=rs[:, :, hp2:hp2 + u],
                                    in0=rs[:, :, k:k + u],
                                    scalar=-1.0,
                                    in1=rs[:, :, k + hn:P],
                                    op0=mybir.AluOpType.mult,
                                    op1=mybir.AluOpType.subtract)
                            if hn > u:
                                # odd leftover N column, negate into place
                                nc.vector.tensor_scalar(
                                    rs[:, :, hp2 + u:hp2 + hn],
                                    rs[:, :, k + u:k + hn],
                                    -1.0, None,
                                    op0=mybir.AluOpType.mult)
                            if sup % GCH == 0:
                                gW = min(GCH, n_sup - sup) * SPC
                                gR = respool.tile([P, GCH * SPC], BF16,
                                                  tag="gR")
                            o0 = (sup % GCH) * SPC
                            with nc.allow_low_precision("bf16 partials"):
                                nc.vector.tensor_reduce(
                                    out=gR[:, o0:o0 + SPC],
                                    in_=rs[:, :, 0:hp2 + hn],
                                    axis=mybir.AxisListType.X,
                                    op=mybir.AluOpType.add)
                            if sup % GCH == GCH - 1 or sup == n_sup - 1:
                                gb = (sup // GCH) * GCH * SPC
                                nc.sync.dma_start(
                                    out=outm[:, gb:gb + gW],
                                    in_=gR[:, 0:gW])
    _strip_duplicate_ldweights(nc)
    nc.compile()
    return nc


def _prep_weights(W1, b1, W2, m_res=M_RES):
    """Fold |W2| into W1, order j-columns, scale, quantize to fp8 + residual.

    Returns (w8 [128,2,128], r8 [128,2,m], k_pos, m_p, m_n, S, order).
    j order: [P-zone asc |w2| | N-zone desc |w2|] -> top-m block is the
    contiguous mid-range [k-mP, k+mN).
    """
    import ml_dtypes

    e4 = ml_dtypes.float8_e4m3
    H = HIDDEN
    W1 = np.asarray(W1, np.float32)
    W2 = np.asarray(W2, np.float32).reshape(-1)
    absw2 = np.abs(W2)
    sgn = np.sign(W2)

    pos = np.nonzero(sgn >= 0)[0]
    neg = np.nonzero(sgn < 0)[0]
    pos = pos[np.argsort(absw2[pos], kind="stable")]          # asc
    neg = neg[np.argsort(-absw2[neg], kind="stable")]         # desc
    order = np.concatenate([pos, neg])
    k_pos = len(pos)

    m = min(m_res, HIDDEN)
    if m > 0:
        thresh_idx = np.argsort(-absw2)[:m]
        in_top = np.zeros(HIDDEN, bool)
        in_top[thresh_idx] = True
        m_p = int(in_top[pos].sum())
        m_n = m - m_p
        # keep the block inside [0, 128)
        if k_pos - m_p < 0:
            m_p = k_pos
            m_n = m - m_p
        if k_pos + m_n > HIDDEN:
            m_n = HIDDEN - k_pos
            m_p = m - m_n
    else:
        m_p = m_n = 0

    wa = (W1[:, :H] * absw2[:, None]).T[:, order]   # [128 in, 128 j]
    wb = (W1[:, H:] * absw2[:, None]).T[:, order]
    wcat = np.stack([wa, wb], 0)                    # [2, 128 in, 128 j]
    wmax = float(np.abs(wcat).max())
    S = 2.0 ** math.floor(math.log2(224.0 / max(wmax, 1e-30)))
    w_s = (wcat * S).astype(np.float32)
    w8 = w_s.astype(e4)
    r_f = w_s - w8.astype(np.float32)
    c0 = k_pos - m_p
    r8 = r_f[:, :, c0:c0 + max(m, 1)].astype(e4)
    # engine layout: [128 in-part, 2 slab, 128 j]
    w8 = np.ascontiguousarray(w8.transpose(1, 0, 2))
    r8 = np.ascontiguousarray(r8.transpose(1, 0, 2))
    return w8, r8, k_pos, m_p, m_n, S, order


def _host_prep(z, pot_arcs, W1, b1, W2, b2, m_res=M_RES):
    """Stage inputs: fold/scale/quantize weights, expand+interleave z."""
    import ml_dtypes

    e4 = ml_dtypes.float8_e4m3
    z = np.asarray(z, np.float32)
    b1 = np.asarray(b1, np.float32).reshape(-1)
    b2 = np.asarray(b2, np.float32).reshape(-1)
    arcs = np.asarray(pot_arcs)
    assert not np.any(b1), "b1 folding not implemented (b1 == 0 in this task)"

    w8, r8, k_pos, m_p, m_n, S, order = _prep_weights(W1, b1, W2, m_res)
    w8_flat = w8.reshape(P, -1)
    r8_flat = r8.reshape(P, -1)

    zT = np.ascontiguousarray(z.T)  # [128, N] f32
    src = np.asarray(arcs[:, 0], np.int64)
    dst = np.asarray(arcs[:, 1], np.int64)
    in_maps = []
    for ci in range(N_CORES):
        lo, hi = ci * E_PER_CORE, (ci + 1) * E_PER_CORE
        s_idx = np.zeros(E_PAD, np.int64)
        d_idx = np.zeros(E_PAD, np.int64)
        s_idx[:E_PER_CORE] = src[lo:hi]
        d_idx[:E_PER_CORE] = dst[lo:hi]
        zs = zT[:, s_idx].astype(e4).reshape(P, NT, P)
        zd = zT[:, d_idx].astype(e4).reshape(P, NT, P)
        # SwInterleave stationary layout: [p, t, arc_rev, (zs, zd)]
        zc = np.stack([zs[:, :, ::-1], zd[:, :, ::-1]], axis=3)
        in_maps.append(dict(
            zc=np.ascontiguousarray(zc).reshape(P, NT * 2 * P),
            w8=w8_flat, r8=r8_flat))
    return in_maps, float(b2[0]), k_pos, m_p, m_n, S


def _assemble(results, b2_val, S):
    """results[c]["outm"] is [128, NT] bf16: signed zone sums."""
    out = np.empty(N_ARCS, np.float32)
    for c in range(N_CORES):
        val = np.asarray(results[c]["outm"], np.float32)
        out[c * E_PER_CORE:(c + 1) * E_PER_CORE] = \
            val.T.reshape(-1)[:E_PER_CORE]
    return out / S + b2_val


_GRAPH_CACHE = {}


def _get_graph(k_pos, m_p, m_n):
    key = (k_pos, m_p, m_n)
    if key not in _GRAPH_CACHE:
        _GRAPH_CACHE[key] = _build_graph(k_pos, m_p, m_n)
    return _GRAPH_CACHE[key]


def _pick_m_res(z, pot_arcs, W1, b1, W2, b2, sample=100000):
    """Host-side emulation of device numerics on an arc sample. Returns the
    smallest residual width m_res clearing the 2e-2 gate with margin.
    """
    import ml_dtypes

    e4 = ml_dtypes.float8_e4m3
    z32 = np.asarray(z, np.float32)
    W1f = np.asarray(W1, np.float32)
    W2f = np.asarray(W2, np.float32).reshape(-1)
    arcs = np.asarray(pot_arcs)[:sample]
    s, d = arcs[:, 0].astype(np.int64), arcs[:, 1].astype(np.int64)
    zs8 = z32[s].astype(e4).astype(np.float32)
    zd8 = z32[d].astype(e4).astype(np.float32)
    h = np.concatenate([z32[s], z32[d]], -1)
    exact = (np.maximum(h @ W1f.T, 0) @ W2f).reshape(-1)
    nex = max(np.linalg.norm(exact), 1e-30)
    sgn = np.sign(W2f)

    for m in (M_RES, 128):
        w8, r8, k_pos, m_p, m_n, S, order = _prep_weights(W1, b1, W2, m)
        wtot = w8.astype(np.float32)            # [128, 2, 128]
        c0 = k_pos - m_p
        if m_p + m_n > 0:
            wtot[:, :, c0:c0 + m_p + m_n] += r8.astype(np.float32)
        q = zs8 @ wtot[:, 0, :] + zd8 @ wtot[:, 1, :]
        rs = np.maximum(q, 0).astype(ml_dtypes.bfloat16).astype(np.float32)
        sg = np.where(sgn[order] >= 0, 1.0, -1.0).astype(np.float32)
        out = (rs * sg).sum(1) / S
        rel = np.linalg.norm(out - exact) / nex
        if rel < 1.9e-2:
            return m
    return 128


def kernel(z, pot_arcs, W1, b1, W2, b2):
    from concourse.bass_utils import run_bass_kernel_spmd

    m_res = _pick_m_res(z, pot_arcs, W1, b1, W2, b2)
    in_maps, b2_val, k_pos, m_p, m_n, S = _host_prep(
        z, pot_arcs, W1, b1, W2, b2, m_res=m_res)
    nc = _get_graph(k_pos, m_p, m_n)
    res = run_bass_kernel_spmd(nc, in_maps, core_ids=list(range(N_CORES)))
    return _assemble(res.results, b2_val, S)


# revision 8
# speedup vs baseline: 1.4929x; 1.4077x over previous
"""ArcDecoder distributed Bass kernel for 8 TRN2 NeuronCores.

Problem: for each arc e with endpoints (s, d):
    h   = concat(z[s], z[d])                # [256]
    h1  = relu(W1 @ h + b1)                 # [128]
    out = W2 @ h1 + b2                      # scalar

Strategy (dense, host-expanded, fp8 DoubleRowSwInterleave): the host
pre-gathers the endpoint embeddings into one interleaved stream per core
in fp8-e4m3, laid out for MatmulPerfMode.DoubleRowSwInterleave (pairs
(zs, zd) interleaved per arc, arcs reversed within each 128-slot tile).
W1 is folded with |W2| and scaled by a power-of-2 S so fp8 quantization
avoids subnormal underflow; j-columns are ordered
[P-zone asc|w2| | N-zone desc|w2|] (k = #positives) so the sign zones
are contiguous and the top-m columns by |w2| form one contiguous
mid-block [k-mP, k+mN) which gets an fp8 residual correction
r8 = fp8(S*w - w8) streamed as a SECOND small matmul.

Per tile, ONE LDWEIGHTS + TWO matmuls (residual first):
    MM1: psum[arc, c0:c0+m] = z_cat.T @ r8   (start=True, N=m small)
    MM2: psum[arc, 0:128]  += z_cat.T @ w8   (stop=True,  N=128)
Both share the same stationary z_cat tile; a post-legalization pass
removes the duplicate LDWEIGHTS (bass emits one per matmul), which
restores the 1-LDW pipeline (~84ns/tile vs 213 with 2 LDWs).

Post-matmul, split across engines (psum chunk = 16 tiles, rs
super-tile = 32 tiles so GPSIMD/DVE instructions amortize overheads):
    relu on ACT (j in [0:X_ACT)) and DVE (rest) -> rs bf16,
    GPSIMD folds each zone's upper half onto its lower half,
    DVE segmented add-reduce of folded halves -> gP, gN (bf16),
    host: out = (gP - gN)/S + b2.
Fully dense streaming: 1MB DMAs, no gathers, no collectives.
"""

import math

import numpy as np

# ---------------- problem constants (hardcoded, per the task spec) ----------
N_NODES = 100000
HIDDEN = 128
N_ARCS = 1000000
N_CORES = 8

P = 128  # SBUF partitions

E_PER_CORE = N_ARCS // N_CORES  # 125000
NT = 992                        # slot tiles per core (992*128 = 126976 slots)
E_PAD = NT * P

PCHUNK = 16   # tiles per psum chunk (16*128 f32 = 8KB/partition = 4 banks)
SUPER = 2     # psum chunks per rs super-tile (32 tiles)
DCHUNK = 32   # tiles per input-DMA chunk (32*256*128 = 1MB fp8)
N_DC = NT // DCHUNK   # 31
N_CH = NT // PCHUNK   # 62

M_RES = 64    # residual-corrected columns (top |w2|), contiguous mid-block
X_ACT = 114   # relu j-split: ACT does [0:x), DVE does [x:128)
FOLD_GPS = 21 # of the 32 super tiles, GPSIMD folds [0:21), DVE folds [21:32)


def _strip_duplicate_ldweights(nc):
    """Remove InstLdweights that reload the exact weights AP the PE already
    holds (consecutive matmuls sharing one stationary tile). bass emits one
    LDWEIGHTS per InstMatmult at legalization; the matmuls themselves are
    non-self-loading, so the duplicate load is pure overhead (~130ns/tile).
    """
    removed = 0
    for blk in nc.main_func.blocks:
        insts = list(blk.instructions)
        keep, last_sig = [], None
        for i in insts:
            tn = type(i).__name__
            if tn == "InstLdweights":
                w = i.ins[0]
                sig = (str(w), getattr(w, "offset", None))
                if sig == last_sig:
                    removed += 1
                    continue
                last_sig = sig
            elif tn != "InstMatmult":
                last_sig = None
            keep.append(i)
        if len(keep) != len(insts):
            blk.instructions = keep
    return removed


def _build_graph(k_pos, m_p, m_n):
    """Build the SPMD single-core graph (all 8 cores run this same graph).

    k_pos: number of leading j-columns with sgn(W2) = +1 (rest negative).
    m_p, m_n: residual block spans psum columns [k_pos-m_p, k_pos+m_n).
    """
    import concourse.bass as bass
    from concourse import bacc, mybir, tile

    BF16 = mybir.dt.bfloat16
    F32 = mybir.dt.float32
    E4 = mybir.dt.float8e4
    DRS = mybir.MatmulPerfMode.DoubleRowSwInterleave

    k = k_pos
    m = m_p + m_n
    c0 = k - m_p  # residual block start column
    hp = (k + 1) // 2
    hn = (P - k + 1) // 2
    x = X_ACT
    SPC = SUPER * PCHUNK  # tiles per rs super-tile

    nc = bacc.Bacc(None, target_bir_lowering=False)
    with tile.TileContext(nc) as tc:
        with tc.tile_pool(name="dram", bufs=1, space="DRAM") as dram:
            zc_d = dram.tile([P, NT * 2 * P], E4, kind="ExternalInput",
                             name="zc", uniquify=False)
            w8_d = dram.tile([P, 2 * P], E4, kind="ExternalInput",
                             name="w8", uniquify=False)
            r8_d = dram.tile([P, 2 * max(m, 1)], E4, kind="ExternalInput",
                             name="r8", uniquify=False)
            outm = dram.tile([P, NT], BF16, kind="ExternalOutput",
                             name="outm", uniquify=False)

            with tc.tile_pool(name="consts", bufs=1) as cpool:
                w8_s = cpool.tile([P, 2, P], E4, name="w8_s")
                nc.sync.dma_start(
                    out=w8_s[:].rearrange("p a b -> p (a b)"), in_=w8_d[:])
                r8_s = cpool.tile([P, 2, max(m, 1)], E4, name="r8_s")
                nc.sync.dma_start(
                    out=r8_s[:].rearrange("p a b -> p (a b)"), in_=r8_d[:])
                GCH = 4  # supers per result-flush group (4*32=128 cols)

                with tc.tile_pool(name="zin", bufs=3) as zpool, \
                     tc.tile_pool(name="ps", bufs=2, space="PSUM") as pspool, \
                     tc.tile_pool(name="res", bufs=2) as respool, \
                     tc.tile_pool(name="rs", bufs=4) as rspool:
                    gP = gN = rs = None
                    n_sup = N_CH // SUPER
                    for c2 in range(N_DC):
                        zc_t = zpool.tile([P, DCHUNK, P, 2], E4, tag="zc")
                        nc.sync.dma_start(
                            out=zc_t[:].rearrange("p t a b -> p (t a b)"),
                            in_=zc_d[:, c2 * DCHUNK * 2 * P:
                                     (c2 + 1) * DCHUNK * 2 * P])
                        for h in range(DCHUNK // PCHUNK):
                            c = c2 * (DCHUNK // PCHUNK) + h
                            sup, ci = divmod(c, SUPER)
                            ps = pspool.tile([P, PCHUNK, P], F32, tag="ps")
                            for t in range(PCHUNK):
                                tt = h * PCHUNK + t
                                zt = zc_t[:, tt, :, :].rearrange(
                                    "p a b -> p (a b)")
                                if m > 0:
                                    nc.tensor.matmul(ps[:, t, c0:c0 + m],
                                                     lhsT=zt, rhs=r8_s[:],
                                                     start=True, stop=False,
                                                     perf_mode=DRS,
                                                     skip_group_check=True)
                                nc.tensor.matmul(ps[:, t, :],
                                                 lhsT=zt, rhs=w8_s[:],
                                                 start=(m == 0), stop=True,
                                                 perf_mode=DRS,
                                                 skip_group_check=True)
                            # relu: ACT on [0:x), DVE on [x:128)
                            if ci == 0:
                                rs = rspool.tile([P, SPC, P], BF16, tag="rs")
                            rsv = rs[:, ci * PCHUNK:(ci + 1) * PCHUNK, :]
                            nc.scalar.activation(
                                out=rsv[:],
                                in_=ps[:],
                                func=mybir.ActivationFunctionType.Relu)
                            if ci != SUPER - 1:
                                continue
                            # per super-tile: folds (GPSIMD) + reduces (DVE)
                            u = P - k - hn  # N-zone upper-half width
                            # GPSIMD: P-zone fold (plain add)
                            if k > 1:
                                nc.gpsimd.tensor_tensor(
                                    out=rs[:, :, 0:k - hp],
                                    in0=rs[:, :, 0:k - hp],
                                    in1=rs[:, :, hp:k],
                                    op=mybir.AluOpType.add)
                            # DVE: N-zone negating fold into [hp, hp+u)
                            if u > 0:
                                nc.vector.scalar_tensor_tensor(
                                    out=rs[:, :, hp:hp + u],
                                    in0=rs[:, :, k:k + u],
                                    scalar=-1.0,
                                    in1=rs[:, :, k + hn:P],
                                    op0=mybir.AluOpType.mult,
                                    op1=mybir.AluOpType.subtract)
                            if sup % GCH == 0:
                                gW = min(GCH, n_sup - sup) * SPC
                                gR = respool.tile([P, GCH * SPC], BF16,
                                                  tag="gR")
                            o0 = (sup % GCH) * SPC
                            with nc.allow_low_precision("bf16 partials"):
                                nc.vector.tensor_reduce(
                                    out=gR[:, o0:o0 + SPC],
                                    in_=rs[:, :, 0:hp + u],
                                    axis=mybir.AxisListType.X,
                                    op=mybir.AluOpType.add)
                                if hn > u:
                                    # odd leftover N column: subtract directly
                                    nc.vector.tensor_tensor(
                                        out=gR[:, o0:o0 + SPC],
                                        in0=gR[:, o0:o0 + SPC],
                                        in1=rs[:, :, k + u].squeeze(),
                                        op=mybir.AluOpType.subtract)
                            if sup % GCH == GCH - 1 or sup == n_sup - 1:
                                gb = (sup // GCH) * GCH * SPC
                                nc.sync.dma_start(
                                    out=outm[:, gb:gb + gW],
                                    in_=gR[:, 0:gW])
    _strip_duplicate_ldweights(nc)
    nc.compile()
    return nc


def _prep_weights(W1, b1, W2, m_res=M_RES):
    """Fold |W2| into W1, order j-columns, scale, quantize to fp8 + residual.

    Returns (w8 [128,2,128], r8 [128,2,m], k_pos, m_p, m_n, S, order).
    j order: [P-zone asc |w2| | N-zone desc |w2|] -> top-m block is the
    contiguous mid-range [k-mP, k+mN).
    """
    import ml_dtypes

    e4 = ml_dtypes.float8_e4m3
    H = HIDDEN
    W1 = np.asarray(W1, np.float32)
    W2 = np.asarray(W2, np.float32).reshape(-1)
    absw2 = np.abs(W2)
    sgn = np.sign(W2)

    pos = np.nonzero(sgn >= 0)[0]
    neg = np.nonzero(sgn < 0)[0]
    pos = pos[np.argsort(absw2[pos], kind="stable")]          # asc
    neg = neg[np.argsort(-absw2[neg], kind="stable")]         # desc
    order = np.concatenate([pos, neg])
    k_pos = len(pos)

    m = min(m_res, HIDDEN)
    if m > 0:
        thresh_idx = np.argsort(-absw2)[:m]
        in_top = np.zeros(HIDDEN, bool)
        in_top[thresh_idx] = True
        m_p = int(in_top[pos].sum())
        m_n = m - m_p
        # keep the block inside [0, 128)
        if k_pos - m_p < 0:
            m_p = k_pos
            m_n = m - m_p
        if k_pos + m_n > HIDDEN:
            m_n = HIDDEN - k_pos
            m_p = m - m_n
    else:
        m_p = m_n = 0

    wa = (W1[:, :H] * absw2[:, None]).T[:, order]   # [128 in, 128 j]
    wb = (W1[:, H:] * absw2[:, None]).T[:, order]
    wcat = np.stack([wa, wb], 0)                    # [2, 128 in, 128 j]
    wmax = float(np.abs(wcat).max())
    S = 2.0 ** math.floor(math.log2(224.0 / max(wmax, 1e-30)))
    w_s = (wcat * S).astype(np.float32)
    w8 = w_s.astype(e4)
    r_f = w_s - w8.astype(np.float32)
    c0 = k_pos - m_p
    r8 = r_f[:, :, c0:c0 + max(m, 1)].astype(e4)
    # engine layout: [128 in-part, 2 slab, 128 j]
    w8 = np.ascontiguousarray(w8.transpose(1, 0, 2))
    r8 = np.ascontiguousarray(r8.transpose(1, 0, 2))
    return w8, r8, k_pos, m_p, m_n, S, order


def _host_prep(z, pot_arcs, W1, b1, W2, b2, m_res=M_RES):
    """Stage inputs: fold/scale/quantize weights, expand+interleave z."""
    import ml_dtypes

    e4 = ml_dtypes.float8_e4m3
    z = np.asarray(z, np.float32)
    b1 = np.asarray(b1, np.float32).reshape(-1)
    b2 = np.asarray(b2, np.float32).reshape(-1)
    arcs = np.asarray(pot_arcs)
    assert not np.any(b1), "b1 folding not implemented (b1 == 0 in this task)"

    w8, r8, k_pos, m_p, m_n, S, order = _prep_weights(W1, b1, W2, m_res)
    w8_flat = w8.reshape(P, -1)
    r8_flat = r8.reshape(P, -1)

    zT = np.ascontiguousarray(z.T)  # [128, N] f32
    src = np.asarray(arcs[:, 0], np.int64)
    dst = np.asarray(arcs[:, 1], np.int64)
    in_maps = []
    for ci in range(N_CORES):
        lo, hi = ci * E_PER_CORE, (ci + 1) * E_PER_CORE
        s_idx = np.zeros(E_PAD, np.int64)
        d_idx = np.zeros(E_PAD, np.int64)
        s_idx[:E_PER_CORE] = src[lo:hi]
        d_idx[:E_PER_CORE] = dst[lo:hi]
        zs = zT[:, s_idx].astype(e4).reshape(P, NT, P)
        zd = zT[:, d_idx].astype(e4).reshape(P, NT, P)
        # SwInterleave stationary layout: [p, t, arc_rev, (zs, zd)]
        zc = np.stack([zs[:, :, ::-1], zd[:, :, ::-1]], axis=3)
        in_maps.append(dict(
            zc=np.ascontiguousarray(zc).reshape(P, NT * 2 * P),
            w8=w8_flat, r8=r8_flat))
    return in_maps, float(b2[0]), k_pos, m_p, m_n, S


def _assemble(results, b2_val, S):
    """results[c]["outm"] is [128, NT] bf16: signed zone sums."""
    out = np.empty(N_ARCS, np.float32)
    for c in range(N_CORES):
        val = np.asarray(results[c]["outm"], np.float32)
        out[c * E_PER_CORE:(c + 1) * E_PER_CORE] = \
            val.T.reshape(-1)[:E_PER_CORE]
    return out / S + b2_val


_GRAPH_CACHE = {}


def _get_graph(k_pos, m_p, m_n):
    key = (k_pos, m_p, m_n)
    if key not in _GRAPH_CACHE:
        _GRAPH_CACHE[key] = _build_graph(k_pos, m_p, m_n)
    return _GRAPH_CACHE[key]


def _pick_m_res(z, pot_arcs, W1, b1, W2, b2, sample=100000):
    """Host-side emulation of device numerics on an arc sample. Returns the
    smallest residual width m_res clearing the 2e-2 gate with margin.
    """
    import ml_dtypes

    e4 = ml_dtypes.float8_e4m3
    z32 = np.asarray(z, np.float32)
    W1f = np.asarray(W1, np.float32)
    W2f = np.asarray(W2, np.float32).reshape(-1)
    arcs = np.asarray(pot_arcs)[:sample]
    s, d = arcs[:, 0].astype(np.int64), arcs[:, 1].astype(np.int64)
    zs8 = z32[s].astype(e4).astype(np.float32)
    zd8 = z32[d].astype(e4).astype(np.float32)
    h = np.concatenate([z32[s], z32[d]], -1)
    exact = (np.maximum(h @ W1f.T, 0) @ W2f).reshape(-1)
    nex = max(np.linalg.norm(exact), 1e-30)
    sgn = np.sign(W2f)

    for m in (M_RES, 128):
        w8, r8, k_pos, m_p, m_n, S, order = _prep_weights(W1, b1, W2, m)
        wtot = w8.astype(np.float32)            # [128, 2, 128]
        c0 = k_pos - m_p
        if m_p + m_n > 0:
            wtot[:, :, c0:c0 + m_p + m_n] += r8.astype(np.float32)
        q = zs8 @ wtot[:, 0, :] + zd8 @ wtot[:, 1, :]
        rs = np.maximum(q, 0).astype(ml_dtypes.bfloat16).astype(np.float32)
        sg = np.where(sgn[order] >= 0, 1.0, -1.0).astype(np.float32)
        out = (rs * sg).sum(1) / S
        rel = np.linalg.norm(out - exact) / nex
        if rel < 1.9e-2:
            return m
    return 128


def kernel(z, pot_arcs, W1, b1, W2, b2):
    from concourse.bass_utils import run_bass_kernel_spmd

    m_res = _pick_m_res(z, pot_arcs, W1, b1, W2, b2)
    in_maps, b2_val, k_pos, m_p, m_n, S = _host_prep(
        z, pot_arcs, W1, b1, W2, b2, m_res=m_res)
    nc = _get_graph(k_pos, m_p, m_n)
    res = run_bass_kernel_spmd(nc, in_maps, core_ids=list(range(N_CORES)))
    return _assemble(res.results, b2_val, S)
